# revision 33
# baseline (speedup 1.0000x reference)
"""Trainium2 Bass kernel for CausalWaveletFieldAttention.

Full-input contract: kernel(**inputs) takes the complete (unsharded) numpy
inputs and returns the full [8, 2048, 1024] float32 output.

Sharding: pure data-parallel over batch B=8 -> one batch element per
NeuronCore (8 cores), zero collectives (the head-coupling einsum mixes heads
within a batch element only).

Per-core pipeline (x pre-transposed to feature-major on host; bf16 compute
with fp32 PSUM accumulation; fp8 DoubleRow where precision allows):
  1. k = x @ Wk.T in fp8 DoubleRow; k2 = Square(k + bk) (ScalarE, fp8 out);
     per-head sums of 64 partitions via an fp8-DR selector matmul ->
     kmag = sqrt(.) (ScalarE), replicated to 128 partitions.
  2. v = x @ Wv.T (bf16, d-major output channels c~ = d*16 + h);
     vb = Identity(v + bv) (ScalarE, psum->sbuf); field = vb * kmag
     (VectorE tensor_tensor, 2x mode).
  3. gate = Sigmoid(x @ Wg.T + bg) in fp8 DoubleRow (precision-checked:
     adds ~3e-3 rel err; gate tile reuses the x buffer, dead after v).
  4. causal multi-scale dilated conv, 22 distinct time offsets split
     across engines by measured cost:
       - offset 0: VectorE tensor_scalar_mul (4x mode) initializes acc
       - offsets 1..24 (9): ScalarE Copy-with-scale into a scratch row +
         VectorE tensor_tensor add (2x mode) into acc
       - offsets 32..256 (7): TensorE matmuls with the head-coupling
         matrix row-scaled by the conv weight (coupling fused in)
       - offsets 384..1536 (5, narrow): VectorE scalar_tensor_tensor
  5. head coupling: in d-major layout coupling is I_8 (x) C^T; it heads
     each (vc, nch) PSUM chain (start=True over the full 512 cols,
     contracting acc), then the TensorE conv offsets accumulate on top.
  6. gated = psum * gate (VectorE), written into the field buffer (dead).
  7. out = gated.T @ Wo.T per token tile; DMA to DRAM.
The tiny softmaxes (scale gains [11,16], coupling [16,16]) are computed
on-device.
"""

import os
import sys

import numpy as np

# recover wedged NeuronCores from a previously killed process
os.environ.setdefault("NEURON_RT_RESET_CORES", "1")

for _p in ("/opt/trn_rl_repo", "/root/.axon_site/_ro/trn_rl_repo"):
    if _p not in sys.path:
        sys.path.append(_p)

import ml_dtypes  # noqa: E402
import concourse.bass as bass  # noqa: E402
import concourse.tile as tile  # noqa: E402
from concourse import bacc, mybir  # noqa: E402
from concourse import bass_utils  # noqa: E402

BF16 = mybir.dt.bfloat16
F32 = mybir.dt.float32
FP8 = mybir.dt.float8e4
NP_BF16 = ml_dtypes.bfloat16
NP_FP8 = ml_dtypes.float8_e4m3

B, N, D = 8, 2048, 1024
H, HD = 16, 64
S = 11  # scales
NCORES = 8
P = 128  # partitions
CH = D // P  # 8 channel chunks
NT = N // P  # 16 token tiles
NCK = N // 512  # 4 free-dim 512 chunks

D4 = np.array(
    [0.4829629131445341, 0.8365163037378079, 0.2241438680420134, -0.1294095225512604],
    dtype=np.float64,
)

# Distinct causal time offsets (3-t)*2^j < N, and the [n_offsets, S] map s.t.
# w[o, h] = sum_j A_MAP[o, j] * softmax_gains[j, h]
_offs = sorted({(3 - t) * (1 << j) for j in range(S) for t in range(4)} & set(range(N)))
OFFSETS = list(_offs)
NOFF = len(OFFSETS)  # 22
A_MAP = np.zeros((NOFF, S), dtype=np.float64)
for j in range(S):
    for t in range(4):
        o = (3 - t) * (1 << j)
        if o < N:
            A_MAP[OFFSETS.index(o), j] += D4[t]

# d-major channel permutation: c~ -> original feature h*64 + d
PERM = np.array([(c % H) * HD + c // H for c in range(D)], dtype=np.int64)

# conv offset -> engine split, balancing measured per-column engine costs
# (PE 0.50 ns, DVE mul+add pair 0.78 ns, ScalarE mul 0.83 + DVE add 0.52,
#  Pool fused MAC 1.98 ns but otherwise idle)
INIT_OFF = 0                                   # DVE tensor_scalar_mul (4x)
ACT_SET = (1, 2, 3)                            # ScalarE mul + DVE add (2x)
DVE_SET = (4, 6, 8, 12, 16)                    # DVE fused MAC
PE_SET = (24, 32, 48, 64, 96, 128, 192, 256, 384)  # TensorE, coupling fused
POOL_SET = (512, 768, 1024, 1536)              # DVE mul + Pool add
INIT_OI = OFFSETS.index(INIT_OFF)
ACT_OFFS = [OFFSETS.index(o) for o in ACT_SET]
DVE_OFFS = [OFFSETS.index(o) for o in DVE_SET]
PE_OFFS = [OFFSETS.index(o) for o in PE_SET]
POOL_OFFS = [OFFSETS.index(o) for o in POOL_SET]
assert {INIT_OI, *ACT_OFFS, *DVE_OFFS, *PE_OFFS, *POOL_OFFS} == set(range(NOFF))

_CACHE = {}


def _build_program(iters=1, ob_zero=True):
    nc = bacc.Bacc("TRN2", target_bir_lowering=False, debug=False, num_devices=NCORES)

    # ---- I/O ----
    x_cm = nc.dram_tensor("x_cm", [D, N], BF16, kind="ExternalInput")
    # fp8 DoubleRow operands: contraction index c = 256*ic + 2*p + j laid
    # out as [p, ic, j, .]
    x8_d = nc.dram_tensor("x8", [P, 4, 2, N], FP8, kind="ExternalInput")
    wk8_d = nc.dram_tensor("wk8", [P, 4, 2, D], FP8, kind="ExternalInput")
    wg8_d = nc.dram_tensor("wg8", [P, 4, 2, D], FP8, kind="ExternalInput")
    wv_d = nc.dram_tensor("wv", [D, D], BF16, kind="ExternalInput")  # [c_in, c~]
    wo_d = nc.dram_tensor("wo", [D, D], BF16, kind="ExternalInput")  # [c~, f]
    bk_d = nc.dram_tensor("bk", [P, CH], F32, kind="ExternalInput")
    bv_d = nc.dram_tensor("bv", [P, CH], F32, kind="ExternalInput")
    bg_d = nc.dram_tensor("bg", [P, CH], F32, kind="ExternalInput")
    sg_d = nc.dram_tensor("sg", [S, H], F32, kind="ExternalInput")
    fc_d = nc.dram_tensor("fc", [H, H], F32, kind="ExternalInput")
    if not ob_zero:
        ob_d = nc.dram_tensor("ob", [P, D], F32, kind="ExternalInput")
    y_d = nc.dram_tensor("y", [N, D], F32, kind="ExternalOutput")

    # ---- constants (embedded in NEFF) ----
    a_rep = np.zeros((H, S, NOFF), dtype=np.float32)
    for hh in range(H):
        a_rep[hh] = A_MAP.T.astype(np.float32)
    a_rep_d = nc.inline_tensor(np.ascontiguousarray(a_rep), "a_rep")
    # fp8 DoubleRow selector: contraction (p, j) over chunk pairs kc=2g+j;
    # head of k-feature 128*kc + p is 2*kc + p//HD
    sel8 = np.zeros((P, 4, 2, H), dtype=NP_FP8)
    for g in range(4):
        for j in range(2):
            kc = 2 * g + j
            for p in range(P):
                sel8[p, g, j, 2 * kc + p // HD] = 1
    sel8_d = nc.inline_tensor(np.ascontiguousarray(sel8), "sel8")
    i16_d = nc.inline_tensor(np.eye(H, dtype=NP_BF16), "i16")

    import contextlib
    with tile.TileContext(nc) as tc, contextlib.ExitStack() as _st:
      for _it in range(iters):
          with (
              tc.tile_pool(name="consts", bufs=1) as cpool,
              tc.tile_pool(name="xpool", bufs=1) as xpool,
              tc.tile_pool(name="x8pool", bufs=1) as x8pool,
              tc.tile_pool(name="w8pool", bufs=1) as w8pool,
              tc.tile_pool(name="wpool", bufs=2) as wpool,
              tc.tile_pool(name="field", bufs=1) as fpool,
              tc.tile_pool(name="accp", bufs=1) as apool,
              tc.tile_pool(name="k2p", bufs=1) as k2pool,
              tc.tile_pool(name="vbp", bufs=3) as vbpool,
              tc.tile_pool(name="tmpp", bufs=10) as tmppool,
              tc.tile_pool(name="ystg", bufs=2) as ypool,
              tc.tile_pool(name="psum", bufs=6, space="PSUM") as pspool,
              tc.tile_pool(name="psum_km", bufs=2, space="PSUM") as kmpool,
          ):
              # ============ big streaming inputs first (head latency) ======
              x_sb = xpool.tile([P, CH, N], BF16, tag="xg")
              x8_sb = x8pool.tile([P, 4, 2, N], FP8)
              wk8_sb = w8pool.tile([P, 4, 2, D], FP8, tag="wk8")
              wg8_sb = w8pool.tile([P, 4, 2, D], FP8, tag="wg8")
              nc.sync.dma_start(out=wk8_sb[:, :, :, :], in_=wk8_d[:, :, :, :])
              nc.sync.dma_start(out=x8_sb[:, :, :, :], in_=x8_d[:, :, :, :])
              nc.sync.dma_start(out=wg8_sb[:, :, :, :], in_=wg8_d[:, :, :, :])
              for ic in range(CH):
                  nc.sync.dma_start(out=x_sb[:, ic, :], in_=x_cm[P * ic:P * (ic + 1), :])
              wv_sb = wpool.tile([P, CH, D], BF16, tag="wmat")
              for ic in range(CH):
                  nc.sync.dma_start(out=wv_sb[:, ic, :], in_=wv_d[P * ic:P * (ic + 1), :])
              wo_sb = wpool.tile([P, CH, D], BF16, tag="wmat")
              for ic in range(CH):
                  nc.sync.dma_start(out=wo_sb[:, ic, :], in_=wo_d[P * ic:P * (ic + 1), :])

              # ============ tiny parameter prep ============
              # softmax of scale_gain over scales, per head -> gains [16, 11]
              sg_sb = cpool.tile([H, S], F32)
              nc.gpsimd.dma_start(out=sg_sb[:, :], in_=sg_d.ap().rearrange("j h -> h j"))
              sg_mx = cpool.tile([H, 1], F32)
              nc.vector.reduce_max(out=sg_mx[:, :], in_=sg_sb[:, :], axis=mybir.AxisListType.X)
              nc.vector.tensor_scalar_mul(sg_mx[:, :], sg_mx[:, :], -1.0)
              sg_e = cpool.tile([H, S], F32)
              nc.scalar.activation(
                  out=sg_e[:, :], in_=sg_sb[:, :],
                  func=mybir.ActivationFunctionType.Exp, bias=sg_mx[:, 0:1], scale=1.0,
              )
              sg_sum = cpool.tile([H, 1], F32)
              nc.vector.reduce_sum(out=sg_sum[:, :], in_=sg_e[:, :], axis=mybir.AxisListType.X)
              sg_rec = cpool.tile([H, 1], F32)
              nc.vector.reciprocal(out=sg_rec[:, :], in_=sg_sum[:, :])
              gains = cpool.tile([H, S], F32)
              nc.vector.tensor_scalar_mul(gains[:, :], sg_e[:, :], sg_rec[:, 0:1])

              # conv coefficients w[h, o] = sum_j gains[h, j] * A_MAP[o, j]
              a_sb = cpool.tile([H, S, NOFF], F32)
              nc.gpsimd.dma_start(out=a_sb[:, :, :], in_=a_rep_d[:, :, :])
              w_sb = cpool.tile([H, NOFF], F32)
              nc.vector.tensor_scalar_mul(w_sb[:, :], a_sb[:, 0, :], gains[:, 0:1])
              for j in range(1, S):
                  nc.vector.scalar_tensor_tensor(
                      out=w_sb[:, :], in0=a_sb[:, j, :], scalar=gains[:, j:j + 1],
                      in1=w_sb[:, :], op0=mybir.AluOpType.mult, op1=mybir.AluOpType.add,
                  )
              # replicate to all 128 partitions (p -> p mod 16)
              w_rep = cpool.tile([P, NOFF], F32)
              for r in range(P // H):
                  nc.gpsimd.dma_start(out=w_rep[H * r:H * (r + 1), :], in_=w_sb[:, :])

              # coupling softmax (rows) -> C_sm; G = I_8 (x) C_sm^T  [128,128] bf16
              fc_sb = cpool.tile([H, H], F32)
              nc.gpsimd.dma_start(out=fc_sb[:, :], in_=fc_d[:, :])
              fc_mx = cpool.tile([H, 1], F32)
              nc.vector.reduce_max(out=fc_mx[:, :], in_=fc_sb[:, :], axis=mybir.AxisListType.X)
              nc.vector.tensor_scalar_mul(fc_mx[:, :], fc_mx[:, :], -1.0)
              fc_e = cpool.tile([H, H], F32)
              nc.scalar.activation(
                  out=fc_e[:, :], in_=fc_sb[:, :],
                  func=mybir.ActivationFunctionType.Exp, bias=fc_mx[:, 0:1], scale=1.0,
              )
              fc_sum = cpool.tile([H, 1], F32)
              nc.vector.reduce_sum(out=fc_sum[:, :], in_=fc_e[:, :], axis=mybir.AxisListType.X)
              fc_rec = cpool.tile([H, 1], F32)
              nc.vector.reciprocal(out=fc_rec[:, :], in_=fc_sum[:, :])
              csm_bf = cpool.tile([H, H], BF16)
              nc.vector.tensor_scalar_mul(csm_bf[:, :], fc_e[:, :], fc_rec[:, 0:1])
              i16_sb = cpool.tile([H, H], BF16)
              nc.gpsimd.dma_start(out=i16_sb[:, :], in_=i16_d[:, :])
              ct_ps = kmpool.tile([H, H], BF16, tag="km")
              nc.tensor.transpose(out=ct_ps[:, :], in_=csm_bf[:, :], identity=i16_sb[:, :])
              ct_bf = cpool.tile([H, H], BF16)
              nc.vector.tensor_copy(ct_bf[:, :], ct_ps[:, :])
              g_sb = cpool.tile([P, P], BF16)
              nc.vector.memset(g_sb[:, :], 0.0)
              for r in range(CH):
                  nc.sync.dma_start(
                      out=g_sb[H * r:H * (r + 1), H * r:H * (r + 1)], in_=ct_bf[:, :]
                  )
              # coupling row-scaled by conv weight for the PE conv offsets
              gdiagw = cpool.tile([P, len(PE_OFFS), P], BF16)
              for gi, oi in enumerate(PE_OFFS):
                  nc.vector.tensor_scalar_mul(
                      gdiagw[:, gi, :], g_sb[:, :], w_rep[:, oi:oi + 1]
                  )

              sel8_sb = cpool.tile([P, 4, 2, H], FP8)
              nc.gpsimd.dma_start(out=sel8_sb[:, :, :, :], in_=sel8_d[:, :, :, :])
              bk_sb = cpool.tile([P, CH], F32)
              nc.gpsimd.dma_start(out=bk_sb[:, :], in_=bk_d[:, :])
              bv_sb = cpool.tile([P, CH], F32)
              nc.gpsimd.dma_start(out=bv_sb[:, :], in_=bv_d[:, :])
              bg_sb = cpool.tile([P, CH], F32)
              nc.gpsimd.dma_start(out=bg_sb[:, :], in_=bg_d[:, :])
              if not ob_zero:
                  ob_sb = cpool.tile([P, D], F32)
                  nc.gpsimd.dma_start(out=ob_sb[:, :], in_=ob_d[:, :])

              # ============ k phase: kmag[h, n] ============
              kmag16 = cpool.tile([H, N], BF16)
              for nch in range(NCK):
                  ns = 512 * nch
                  k2 = k2pool.tile([P, CH, 512], FP8, tag="k2")
                  for kc in range(CH):
                      ps = pspool.tile([P, 512], F32, tag="mm")
                      for ic in range(4):
                          nc.tensor.matmul(
                              ps[:, :],
                              lhsT=wk8_sb[:, ic, :, P * kc:P * (kc + 1)],
                              rhs=x8_sb[:, ic, :, ns:ns + 512],
                              perf_mode=mybir.MatmulPerfMode.DoubleRow,
                              start=(ic == 0), stop=(ic == 3),
                          )
                      nc.scalar.activation(
                          out=k2[:, kc, :], in_=ps[:, :],
                          func=mybir.ActivationFunctionType.Square,
                          bias=bk_sb[:, kc:kc + 1], scale=1.0,
                      )
                  km_ps = kmpool.tile([H, 512], F32, tag="km")
                  for g in range(4):
                      nc.tensor.matmul(
                          km_ps[:, :],
                          lhsT=sel8_sb[:, g, :, :],
                          rhs=k2[:, 2 * g:2 * g + 2, :],
                          perf_mode=mybir.MatmulPerfMode.DoubleRow,
                          start=(g == 0), stop=(g == 3),
                      )
                  nc.scalar.activation(
                      out=kmag16[:, ns:ns + 512], in_=km_ps[:, :],
                      func=mybir.ActivationFunctionType.Sqrt,
                  )
              kmag_rep = cpool.tile([P, N], BF16)
              for r in range(P // H):
                  nc.sync.dma_start(out=kmag_rep[H * r:H * (r + 1), :], in_=kmag16[:, :])

              # ============ v phase + conv (DVE/Act offsets), per vc ======
              field = fpool.tile([P, CH, N], BF16)
              acc = apool.tile([P, CH, N], BF16)
              for vc in range(CH):
                  for nch in range(NCK):
                      ns = 512 * nch
                      ps = pspool.tile([P, 512], F32, tag="mm")
                      for ic in range(CH):
                          nc.tensor.matmul(
                              ps[:, :],
                              lhsT=wv_sb[:, ic, P * vc:P * (vc + 1)],
                              rhs=x_sb[:, ic, ns:ns + 512],
                              start=(ic == 0), stop=(ic == CH - 1),
                          )
                      vb = vbpool.tile([P, 512], BF16, tag="vb")
                      with tc.high_priority():
                          nc.scalar.activation(
                              out=vb[:, :], in_=ps[:, :],
                              func=mybir.ActivationFunctionType.Identity,
                              bias=bv_sb[:, vc:vc + 1], scale=1.0,
                          )
                      with tc.high_priority():
                          nc.vector.tensor_tensor(
                              out=field[:, vc, ns:ns + 512], in0=vb[:, :],
                              in1=kmag_rep[:, ns:ns + 512], op=mybir.AluOpType.mult,
                          )

              # --- conv on DVE/Act/Pool, SEGMENT-MAJOR so acc[:, :, seg]
              # completes in wave order and the coupling chains + out
              # projection pipeline behind the conv train ---
              for cs in range(NCK):
                  lo_s, hi_s = 512 * cs, 512 * (cs + 1)
                  for vc in range(CH):
                      # offset 0 initializes this segment (4x mode)
                      nc.vector.tensor_scalar_mul(
                          acc[:, vc, lo_s:hi_s], field[:, vc, lo_s:hi_s],
                          w_rep[:, INIT_OI:INIT_OI + 1],
                      )
                      # Pool has no per-partition-scalar op (walrus rejects
                      # TensorScalarPtr on Pool): DVE pre-scales into a tmp
                      # (4x mode) and Pool does the accumulate
                      for oi in POOL_OFFS:
                          o = OFFSETS[oi]
                          lo = max(lo_s, o)
                          if lo >= hi_s:
                              continue
                          tmp = tmppool.tile([P, 512], BF16, tag="tmp")
                          nc.vector.tensor_scalar_mul(
                              tmp[:, 0:hi_s - lo],
                              field[:, vc, lo - o:hi_s - o],
                              w_rep[:, oi:oi + 1],
                          )
                          nc.gpsimd.tensor_tensor(
                              out=acc[:, vc, lo:hi_s], in0=acc[:, vc, lo:hi_s],
                              in1=tmp[:, 0:hi_s - lo], op=mybir.AluOpType.add,
                          )
                      for oi in ACT_OFFS:
                          o = OFFSETS[oi]
                          lo = max(lo_s, o)
                          tmp = tmppool.tile([P, 512], BF16, tag="tmp")
                          nc.scalar.activation(
                              out=tmp[:, 0:hi_s - lo], in_=field[:, vc, lo - o:hi_s - o],
                              func=mybir.ActivationFunctionType.Copy,
                              bias=0.0, scale=w_rep[:, oi:oi + 1],
                          )
                          nc.vector.tensor_tensor(
                              out=acc[:, vc, lo:hi_s], in0=acc[:, vc, lo:hi_s],
                              in1=tmp[:, 0:hi_s - lo],
                              op=mybir.AluOpType.add,
                          )
                      for oi in DVE_OFFS:
                          o = OFFSETS[oi]
                          lo = max(lo_s, o)
                          nc.vector.scalar_tensor_tensor(
                              out=acc[:, vc, lo:hi_s],
                              in0=field[:, vc, lo - o:hi_s - o],
                              scalar=w_rep[:, oi:oi + 1],
                              in1=acc[:, vc, lo:hi_s],
                              op0=mybir.AluOpType.mult, op1=mybir.AluOpType.add,
                          )

              # ============ gate phase (d-major channels, fp8 DR) =========
              # gate reuses the x_sb buffer (x dead after the v matmuls)
              gate = xpool.tile([P, CH, N], BF16, tag="xg")
              for gc in range(CH):
                  for nch in range(NCK):
                      ns = 512 * nch
                      ps = pspool.tile([P, 512], F32, tag="mm")
                      for ic in range(4):
                          nc.tensor.matmul(
                              ps[:, :],
                              lhsT=wg8_sb[:, ic, :, P * gc:P * (gc + 1)],
                              rhs=x8_sb[:, ic, :, ns:ns + 512],
                              perf_mode=mybir.MatmulPerfMode.DoubleRow,
                              start=(ic == 0), stop=(ic == 3),
                          )
                      with tc.high_priority():
                          nc.scalar.activation(
                              out=gate[:, gc, ns:ns + 512], in_=ps[:, :],
                              func=mybir.ActivationFunctionType.Sigmoid,
                              bias=bg_sb[:, gc:gc + 1], scale=1.0,
                          )

              # ==== coupling chain + PE conv offsets + gated, segment-outer =
              # gated multiplies the chain PSUM into the gate tile IN PLACE
              # (each gate element is consumed exactly once), so the out
              # projection for segment s can start right after segment s's
              # chains — no end-of-kernel projection tail.
              gated = gate
              for seg in range(NCK):
                  ns = 512 * seg
                  for vc in range(CH):
                      ps = pspool.tile([P, 512], F32, tag="mm")
                      if seg == 0:
                          # low cols are covered only by the coupling matmul,
                          # so it must open the chain (start=True)
                          nc.tensor.matmul(
                              ps[:, :], lhsT=g_sb[:, :],
                              rhs=acc[:, vc, ns:ns + 512],
                              start=True, stop=False,
                          )
                          for gi, oi in enumerate(PE_OFFS):
                              o = OFFSETS[oi]
                              lo = max(0, o - ns)
                              nc.tensor.matmul(
                                  ps[:, lo:512],
                                  lhsT=gdiagw[:, gi, :],
                                  rhs=field[:, vc, ns + lo - o:ns + 512 - o],
                                  start=False, stop=(gi == len(PE_OFFS) - 1),
                              )
                      else:
                          # conv offsets first (they need only field, so the
                          # scheduler can hoist them into earlier PE idle);
                          # the coupling matmul closes the chain once
                          # acc[:, vc, seg] is ready
                          for gi, oi in enumerate(PE_OFFS):
                              nc.tensor.matmul(
                                  ps[:, :],
                                  lhsT=gdiagw[:, gi, :],
                                  rhs=field[:, vc, ns - OFFSETS[oi]:
                                             ns + 512 - OFFSETS[oi]],
                                  start=(gi == 0), stop=False,
                              )
                          nc.tensor.matmul(
                              ps[:, :], lhsT=g_sb[:, :],
                              rhs=acc[:, vc, ns:ns + 512],
                              start=False, stop=True,
                          )
                      with tc.high_priority():
                          nc.vector.tensor_tensor(
                              out=gated[:, vc, ns:ns + 512], in0=ps[:, :],
                              in1=gate[:, vc, ns:ns + 512], op=mybir.AluOpType.mult,
                          )
                  # ---- out projection for this segment's 4 token tiles ----
                  for nt in range(4 * seg, 4 * seg + 4):
                      ystg = ypool.tile([P, D], F32, tag="y")
                      for fch in range(2):
                          fs = 512 * fch
                          ps = pspool.tile([P, 512], F32, tag="mm")
                          for vc in range(CH):
                              nc.tensor.matmul(
                                  ps[:, :],
                                  lhsT=gated[:, vc, P * nt:P * (nt + 1)],
                                  rhs=wo_sb[:, vc, fs:fs + 512],
                                  start=(vc == 0), stop=(vc == CH - 1),
                              )
                          with tc.high_priority():
                              if ob_zero:
                                  nc.scalar.activation(
                                      out=ystg[:, fs:fs + 512], in_=ps[:, :],
                                      func=mybir.ActivationFunctionType.Copy,
                                  )
                              else:
                                  nc.vector.tensor_add(
                                      ystg[:, fs:fs + 512], ps[:, :],
                                      ob_sb[:, fs:fs + 512],
                                  )
                      nc.sync.dma_start(out=y_d[P * nt:P * (nt + 1), :], in_=ystg[:, :])

    nc.compile()
    return nc


def _prep_shared(qkv_w, qkv_b, out_w, out_b, gate_w, gate_b, scale_gain, field_coupling):
    perm = PERM
    wk8 = np.ascontiguousarray(
        qkv_w[D:2 * D, :].T.reshape(4, P, 2, D).transpose(1, 0, 2, 3)
        .astype(NP_FP8))
    wg8 = np.ascontiguousarray(
        gate_w[perm, :].T.reshape(4, P, 2, D).transpose(1, 0, 2, 3)
        .astype(NP_FP8))
    wv = np.ascontiguousarray(qkv_w[2 * D:3 * D, :][perm, :].T.astype(NP_BF16))
    wo = np.ascontiguousarray(out_w[:, perm].T.astype(NP_BF16))
    bk = np.ascontiguousarray(qkv_b[D:2 * D].reshape(CH, P).T.astype(np.float32))
    bv = np.ascontiguousarray(qkv_b[2 * D:3 * D][perm].reshape(CH, P).T.astype(np.float32))
    bg = np.ascontiguousarray(gate_b[perm].reshape(CH, P).T.astype(np.float32))
    ob = np.ascontiguousarray(np.broadcast_to(out_b.astype(np.float32), (P, D)))
    sg = np.ascontiguousarray(scale_gain.astype(np.float32))
    fc = np.ascontiguousarray(field_coupling.astype(np.float32))
    return {"wk8": wk8, "wg8": wg8, "wv": wv, "wo": wo, "bk": bk,
            "bv": bv, "bg": bg, "ob": ob, "sg": sg, "fc": fc}


def _make_in_maps(x, shared, ob_zero=True):
    in_maps = []
    for b in range(B):
        m = dict(shared)
        if ob_zero:
            m.pop("ob", None)
        xt = x[b].T
        m["x_cm"] = np.ascontiguousarray(xt.astype(NP_BF16))
        m["x8"] = np.ascontiguousarray(
            xt.reshape(4, P, 2, N).transpose(1, 0, 2, 3).astype(NP_FP8))
        in_maps.append(m)
    return in_maps


def kernel(x, qkv_w, qkv_b, out_w, out_b, gate_w, gate_b, scale_gain,
           field_coupling):
    x = np.asarray(x, dtype=np.float32)
    qkv_w = np.asarray(qkv_w, dtype=np.float32)
    qkv_b = np.asarray(qkv_b, dtype=np.float32)
    out_w = np.asarray(out_w, dtype=np.float32)
    out_b = np.asarray(out_b, dtype=np.float32)
    gate_w = np.asarray(gate_w, dtype=np.float32)
    gate_b = np.asarray(gate_b, dtype=np.float32)
    scale_gain = np.asarray(scale_gain, dtype=np.float32)
    field_coupling = np.asarray(field_coupling, dtype=np.float32)

    ob_zero = not np.any(out_b)
    key = ("nc", ob_zero)
    if key not in _CACHE:
        _CACHE[key] = _build_program(ob_zero=ob_zero)
    nc = _CACHE[key]

    shared = _prep_shared(qkv_w, qkv_b, out_w, out_b, gate_w, gate_b,
                          scale_gain, field_coupling)
    in_maps = _make_in_maps(x, shared, ob_zero=ob_zero)

    res = bass_utils.run_bass_kernel_spmd(nc, in_maps, list(range(NCORES)))
    out = np.stack([np.asarray(res.results[b]["y"], dtype=np.float32)
                    for b in range(B)], axis=0)
    return out


# revision 35
# speedup vs baseline: 1.9899x; 1.9899x over previous
"""Trainium2 Bass kernel for CausalWaveletFieldAttention.

Full-input contract: kernel(**inputs) takes the complete (unsharded) numpy
inputs and returns the full [8, 2048, 1024] float32 output.

Sharding: pure data-parallel over batch B=8 -> one batch element per
NeuronCore (8 cores), zero collectives (the head-coupling einsum mixes heads
within a batch element only).

Per-core pipeline (x pre-transposed to feature-major on host; bf16 compute
with fp32 PSUM accumulation; fp8 DoubleRow where precision allows):
  1. k = x @ Wk.T in fp8 DoubleRow; k2 = Square(k + bk) (ScalarE, fp8 out);
     per-head sums of 64 partitions via an fp8-DR selector matmul ->
     kmag = sqrt(.) (ScalarE), replicated to 128 partitions.
  2. v = x @ Wv.T (bf16, d-major output channels c~ = d*16 + h);
     vb = Identity(v + bv) (ScalarE, psum->sbuf); field = vb * kmag
     (VectorE tensor_tensor, 2x mode).
  3. gate = Sigmoid(x @ Wg.T + bg) in fp8 DoubleRow (precision-checked:
     adds ~3e-3 rel err; gate tile reuses the x buffer, dead after v).
  4. causal multi-scale dilated conv, 22 distinct time offsets split
     across engines by measured cost:
       - offset 0: VectorE tensor_scalar_mul (4x mode) initializes acc
       - offsets 1..24 (9): ScalarE Copy-with-scale into a scratch row +
         VectorE tensor_tensor add (2x mode) into acc
       - offsets 32..256 (7): TensorE matmuls with the head-coupling
         matrix row-scaled by the conv weight (coupling fused in)
       - offsets 384..1536 (5, narrow): VectorE scalar_tensor_tensor
  5. head coupling: in d-major layout coupling is I_8 (x) C^T; it heads
     each (vc, nch) PSUM chain (start=True over the full 512 cols,
     contracting acc), then the TensorE conv offsets accumulate on top.
  6. gated = psum * gate (VectorE), written into the field buffer (dead).
  7. out = gated.T @ Wo.T per token tile; DMA to DRAM.
The tiny softmaxes (scale gains [11,16], coupling [16,16]) are computed
on-device.
"""

import os
import sys

import numpy as np

# recover wedged NeuronCores from a previously killed process
os.environ.setdefault("NEURON_RT_RESET_CORES", "1")

for _p in ("/opt/trn_rl_repo", "/root/.axon_site/_ro/trn_rl_repo"):
    if _p not in sys.path:
        sys.path.append(_p)

import ml_dtypes  # noqa: E402
import concourse.bass as bass  # noqa: E402
import concourse.tile as tile  # noqa: E402
from concourse import bacc, mybir  # noqa: E402
from concourse import bass_utils  # noqa: E402

BF16 = mybir.dt.bfloat16
F32 = mybir.dt.float32
FP8 = mybir.dt.float8e4
NP_BF16 = ml_dtypes.bfloat16
NP_FP8 = ml_dtypes.float8_e4m3

B, N, D = 8, 2048, 1024
H, HD = 16, 64
S = 11  # scales
NCORES = 8
P = 128  # partitions
CH = D // P  # 8 channel chunks
NT = N // P  # 16 token tiles
NCK = N // 512  # 4 free-dim 512 chunks

D4 = np.array(
    [0.4829629131445341, 0.8365163037378079, 0.2241438680420134, -0.1294095225512604],
    dtype=np.float64,
)

# Distinct causal time offsets (3-t)*2^j < N, and the [n_offsets, S] map s.t.
# w[o, h] = sum_j A_MAP[o, j] * softmax_gains[j, h]
_offs = sorted({(3 - t) * (1 << j) for j in range(S) for t in range(4)} & set(range(N)))
OFFSETS = list(_offs)
NOFF = len(OFFSETS)  # 22
A_MAP = np.zeros((NOFF, S), dtype=np.float64)
for j in range(S):
    for t in range(4):
        o = (3 - t) * (1 << j)
        if o < N:
            A_MAP[OFFSETS.index(o), j] += D4[t]

# d-major channel permutation: c~ -> original feature h*64 + d
PERM = np.array([(c % H) * HD + c // H for c in range(D)], dtype=np.int64)

# conv offset -> engine split, balancing measured per-column engine costs
# (PE 0.50 ns, DVE mul+add pair 0.78 ns, ScalarE mul 0.83 + DVE add 0.52,
#  Pool fused MAC 1.98 ns but otherwise idle)
INIT_OFF = 0                                   # DVE tensor_scalar_mul (4x)
ACT_SET = (1, 2, 3)                            # ScalarE mul + DVE add (2x)
DVE_SET = (4, 6, 8, 12, 16, 512, 768, 1024, 1536)  # DVE fused MAC
PE_SET = (24, 32, 48, 64, 96, 128, 192, 256, 384)  # TensorE, coupling fused
POOL_SET = ()                                  # Pool too slow on real HW
INIT_OI = OFFSETS.index(INIT_OFF)
ACT_OFFS = [OFFSETS.index(o) for o in ACT_SET]
DVE_OFFS = [OFFSETS.index(o) for o in DVE_SET]
PE_OFFS = [OFFSETS.index(o) for o in PE_SET]
POOL_OFFS = [OFFSETS.index(o) for o in POOL_SET]
assert {INIT_OI, *ACT_OFFS, *DVE_OFFS, *PE_OFFS, *POOL_OFFS} == set(range(NOFF))

_CACHE = {}


def _build_program(iters=1, ob_zero=True):
    nc = bacc.Bacc("TRN2", target_bir_lowering=False, debug=False, num_devices=NCORES)

    # ---- I/O ----
    x_cm = nc.dram_tensor("x_cm", [D, N], BF16, kind="ExternalInput")
    # fp8 DoubleRow operands: contraction index c = 256*ic + 2*p + j laid
    # out as [p, ic, j, .]
    x8_d = nc.dram_tensor("x8", [P, 4, 2, N], FP8, kind="ExternalInput")
    wk8_d = nc.dram_tensor("wk8", [P, 4, 2, D], FP8, kind="ExternalInput")
    wg8_d = nc.dram_tensor("wg8", [P, 4, 2, D], FP8, kind="ExternalInput")
    wv_d = nc.dram_tensor("wv", [D, D], BF16, kind="ExternalInput")  # [c_in, c~]
    wo_d = nc.dram_tensor("wo", [D, D], BF16, kind="ExternalInput")  # [c~, f]
    bk_d = nc.dram_tensor("bk", [P, CH], F32, kind="ExternalInput")
    bv_d = nc.dram_tensor("bv", [P, CH], F32, kind="ExternalInput")
    bg_d = nc.dram_tensor("bg", [P, CH], F32, kind="ExternalInput")
    sg_d = nc.dram_tensor("sg", [S, H], F32, kind="ExternalInput")
    fc_d = nc.dram_tensor("fc", [H, H], F32, kind="ExternalInput")
    if not ob_zero:
        ob_d = nc.dram_tensor("ob", [P, D], F32, kind="ExternalInput")
    y_d = nc.dram_tensor("y", [N, D], F32, kind="ExternalOutput")

    # ---- constants (embedded in NEFF) ----
    a_rep = np.zeros((H, S, NOFF), dtype=np.float32)
    for hh in range(H):
        a_rep[hh] = A_MAP.T.astype(np.float32)
    a_rep_d = nc.inline_tensor(np.ascontiguousarray(a_rep), "a_rep")
    # fp8 DoubleRow selector: contraction (p, j) over chunk pairs kc=2g+j;
    # head of k-feature 128*kc + p is 2*kc + p//HD
    sel8 = np.zeros((P, 4, 2, H), dtype=NP_FP8)
    for g in range(4):
        for j in range(2):
            kc = 2 * g + j
            for p in range(P):
                sel8[p, g, j, 2 * kc + p // HD] = 1
    sel8_d = nc.inline_tensor(np.ascontiguousarray(sel8), "sel8")
    i16_d = nc.inline_tensor(np.eye(H, dtype=NP_BF16), "i16")

    import contextlib
    with tile.TileContext(nc) as tc, contextlib.ExitStack() as _st:
      for _it in range(iters):
          with (
              tc.tile_pool(name="consts", bufs=1) as cpool,
              tc.tile_pool(name="xpool", bufs=1) as xpool,
              tc.tile_pool(name="x8pool", bufs=1) as x8pool,
              tc.tile_pool(name="w8pool", bufs=1) as w8pool,
              tc.tile_pool(name="wpool", bufs=2) as wpool,
              tc.tile_pool(name="field", bufs=1) as fpool,
              tc.tile_pool(name="accp", bufs=1) as apool,
              tc.tile_pool(name="k2p", bufs=1) as k2pool,
              tc.tile_pool(name="vbp", bufs=3) as vbpool,
              tc.tile_pool(name="tmpp", bufs=10) as tmppool,
              tc.tile_pool(name="ystg", bufs=2) as ypool,
              tc.tile_pool(name="psum", bufs=6, space="PSUM") as pspool,
              tc.tile_pool(name="psum_km", bufs=2, space="PSUM") as kmpool,
          ):
              # ============ big streaming inputs first (head latency) ======
              x_sb = xpool.tile([P, CH, N], BF16, tag="xg")
              x8_sb = x8pool.tile([P, 4, 2, N], FP8)
              wk8_sb = w8pool.tile([P, 4, 2, D], FP8, tag="wk8")
              wg8_sb = w8pool.tile([P, 4, 2, D], FP8, tag="wg8")
              nc.sync.dma_start(out=wk8_sb[:, :, :, :], in_=wk8_d[:, :, :, :])
              nc.sync.dma_start(out=x8_sb[:, :, :, :], in_=x8_d[:, :, :, :])
              nc.sync.dma_start(out=wg8_sb[:, :, :, :], in_=wg8_d[:, :, :, :])
              for ic in range(CH):
                  nc.sync.dma_start(out=x_sb[:, ic, :], in_=x_cm[P * ic:P * (ic + 1), :])
              wv_sb = wpool.tile([P, CH, D], BF16, tag="wmat")
              for ic in range(CH):
                  nc.sync.dma_start(out=wv_sb[:, ic, :], in_=wv_d[P * ic:P * (ic + 1), :])
              wo_sb = wpool.tile([P, CH, D], BF16, tag="wmat")
              for ic in range(CH):
                  nc.sync.dma_start(out=wo_sb[:, ic, :], in_=wo_d[P * ic:P * (ic + 1), :])

              # ============ tiny parameter prep ============
              # softmax of scale_gain over scales, per head -> gains [16, 11]
              sg_sb = cpool.tile([H, S], F32)
              nc.gpsimd.dma_start(out=sg_sb[:, :], in_=sg_d.ap().rearrange("j h -> h j"))
              sg_mx = cpool.tile([H, 1], F32)
              nc.vector.reduce_max(out=sg_mx[:, :], in_=sg_sb[:, :], axis=mybir.AxisListType.X)
              nc.vector.tensor_scalar_mul(sg_mx[:, :], sg_mx[:, :], -1.0)
              sg_e = cpool.tile([H, S], F32)
              nc.scalar.activation(
                  out=sg_e[:, :], in_=sg_sb[:, :],
                  func=mybir.ActivationFunctionType.Exp, bias=sg_mx[:, 0:1], scale=1.0,
              )
              sg_sum = cpool.tile([H, 1], F32)
              nc.vector.reduce_sum(out=sg_sum[:, :], in_=sg_e[:, :], axis=mybir.AxisListType.X)
              sg_rec = cpool.tile([H, 1], F32)
              nc.vector.reciprocal(out=sg_rec[:, :], in_=sg_sum[:, :])
              gains = cpool.tile([H, S], F32)
              nc.vector.tensor_scalar_mul(gains[:, :], sg_e[:, :], sg_rec[:, 0:1])

              # conv coefficients w[h, o] = sum_j gains[h, j] * A_MAP[o, j]
              a_sb = cpool.tile([H, S, NOFF], F32)
              nc.gpsimd.dma_start(out=a_sb[:, :, :], in_=a_rep_d[:, :, :])
              w_sb = cpool.tile([H, NOFF], F32)
              nc.vector.tensor_scalar_mul(w_sb[:, :], a_sb[:, 0, :], gains[:, 0:1])
              for j in range(1, S):
                  nc.vector.scalar_tensor_tensor(
                      out=w_sb[:, :], in0=a_sb[:, j, :], scalar=gains[:, j:j + 1],
                      in1=w_sb[:, :], op0=mybir.AluOpType.mult, op1=mybir.AluOpType.add,
                  )
              # replicate to all 128 partitions (p -> p mod 16)
              w_rep = cpool.tile([P, NOFF], F32)
              for r in range(P // H):
                  nc.gpsimd.dma_start(out=w_rep[H * r:H * (r + 1), :], in_=w_sb[:, :])

              # coupling softmax (rows) -> C_sm; G = I_8 (x) C_sm^T  [128,128] bf16
              fc_sb = cpool.tile([H, H], F32)
              nc.gpsimd.dma_start(out=fc_sb[:, :], in_=fc_d[:, :])
              fc_mx = cpool.tile([H, 1], F32)
              nc.vector.reduce_max(out=fc_mx[:, :], in_=fc_sb[:, :], axis=mybir.AxisListType.X)
              nc.vector.tensor_scalar_mul(fc_mx[:, :], fc_mx[:, :], -1.0)
              fc_e = cpool.tile([H, H], F32)
              nc.scalar.activation(
                  out=fc_e[:, :], in_=fc_sb[:, :],
                  func=mybir.ActivationFunctionType.Exp, bias=fc_mx[:, 0:1], scale=1.0,
              )
              fc_sum = cpool.tile([H, 1], F32)
              nc.vector.reduce_sum(out=fc_sum[:, :], in_=fc_e[:, :], axis=mybir.AxisListType.X)
              fc_rec = cpool.tile([H, 1], F32)
              nc.vector.reciprocal(out=fc_rec[:, :], in_=fc_sum[:, :])
              csm_bf = cpool.tile([H, H], BF16)
              nc.vector.tensor_scalar_mul(csm_bf[:, :], fc_e[:, :], fc_rec[:, 0:1])
              i16_sb = cpool.tile([H, H], BF16)
              nc.gpsimd.dma_start(out=i16_sb[:, :], in_=i16_d[:, :])
              ct_ps = kmpool.tile([H, H], BF16, tag="km")
              nc.tensor.transpose(out=ct_ps[:, :], in_=csm_bf[:, :], identity=i16_sb[:, :])
              ct_bf = cpool.tile([H, H], BF16)
              nc.vector.tensor_copy(ct_bf[:, :], ct_ps[:, :])
              g_sb = cpool.tile([P, P], BF16)
              nc.vector.memset(g_sb[:, :], 0.0)
              for r in range(CH):
                  nc.sync.dma_start(
                      out=g_sb[H * r:H * (r + 1), H * r:H * (r + 1)], in_=ct_bf[:, :]
                  )
              # coupling row-scaled by conv weight for the PE conv offsets
              gdiagw = cpool.tile([P, len(PE_OFFS), P], BF16)
              for gi, oi in enumerate(PE_OFFS):
                  nc.vector.tensor_scalar_mul(
                      gdiagw[:, gi, :], g_sb[:, :], w_rep[:, oi:oi + 1]
                  )

              sel8_sb = cpool.tile([P, 4, 2, H], FP8)
              nc.gpsimd.dma_start(out=sel8_sb[:, :, :, :], in_=sel8_d[:, :, :, :])
              bk_sb = cpool.tile([P, CH], F32)
              nc.gpsimd.dma_start(out=bk_sb[:, :], in_=bk_d[:, :])
              bv_sb = cpool.tile([P, CH], F32)
              nc.gpsimd.dma_start(out=bv_sb[:, :], in_=bv_d[:, :])
              bg_sb = cpool.tile([P, CH], F32)
              nc.gpsimd.dma_start(out=bg_sb[:, :], in_=bg_d[:, :])
              if not ob_zero:
                  ob_sb = cpool.tile([P, D], F32)
                  nc.gpsimd.dma_start(out=ob_sb[:, :], in_=ob_d[:, :])

              # ============ k phase: kmag[h, n] ============
              kmag16 = cpool.tile([H, N], BF16)
              for nch in range(NCK):
                  ns = 512 * nch
                  k2 = k2pool.tile([P, CH, 512], FP8, tag="k2")
                  for kc in range(CH):
                      ps = pspool.tile([P, 512], F32, tag="mm")
                      for ic in range(4):
                          nc.tensor.matmul(
                              ps[:, :],
                              lhsT=wk8_sb[:, ic, :, P * kc:P * (kc + 1)],
                              rhs=x8_sb[:, ic, :, ns:ns + 512],
                              perf_mode=mybir.MatmulPerfMode.DoubleRow,
                              start=(ic == 0), stop=(ic == 3),
                          )
                      nc.scalar.activation(
                          out=k2[:, kc, :], in_=ps[:, :],
                          func=mybir.ActivationFunctionType.Square,
                          bias=bk_sb[:, kc:kc + 1], scale=1.0,
                      )
                  km_ps = kmpool.tile([H, 512], F32, tag="km")
                  for g in range(4):
                      nc.tensor.matmul(
                          km_ps[:, :],
                          lhsT=sel8_sb[:, g, :, :],
                          rhs=k2[:, 2 * g:2 * g + 2, :],
                          perf_mode=mybir.MatmulPerfMode.DoubleRow,
                          start=(g == 0), stop=(g == 3),
                      )
                  nc.scalar.activation(
                      out=kmag16[:, ns:ns + 512], in_=km_ps[:, :],
                      func=mybir.ActivationFunctionType.Sqrt,
                  )
              kmag_rep = cpool.tile([P, N], BF16)
              for r in range(P // H):
                  nc.sync.dma_start(out=kmag_rep[H * r:H * (r + 1), :], in_=kmag16[:, :])

              # ============ v phase + conv (DVE/Act offsets), per vc ======
              field = fpool.tile([P, CH, N], BF16)
              acc = apool.tile([P, CH, N], BF16)
              for vc in range(CH):
                  for nch in range(NCK):
                      ns = 512 * nch
                      ps = pspool.tile([P, 512], F32, tag="mm")
                      for ic in range(CH):
                          nc.tensor.matmul(
                              ps[:, :],
                              lhsT=wv_sb[:, ic, P * vc:P * (vc + 1)],
                              rhs=x_sb[:, ic, ns:ns + 512],
                              start=(ic == 0), stop=(ic == CH - 1),
                          )
                      vb = vbpool.tile([P, 512], BF16, tag="vb")
                      with tc.high_priority():
                          nc.scalar.activation(
                              out=vb[:, :], in_=ps[:, :],
                              func=mybir.ActivationFunctionType.Identity,
                              bias=bv_sb[:, vc:vc + 1], scale=1.0,
                          )
                      with tc.high_priority():
                          nc.vector.tensor_tensor(
                              out=field[:, vc, ns:ns + 512], in0=vb[:, :],
                              in1=kmag_rep[:, ns:ns + 512], op=mybir.AluOpType.mult,
                          )

              # --- conv on DVE/Act/Pool, SEGMENT-MAJOR so acc[:, :, seg]
              # completes in wave order and the coupling chains + out
              # projection pipeline behind the conv train ---
              for cs in range(NCK):
                  lo_s, hi_s = 512 * cs, 512 * (cs + 1)
                  for vc in range(CH):
                      # offset 0 initializes this segment (4x mode)
                      nc.vector.tensor_scalar_mul(
                          acc[:, vc, lo_s:hi_s], field[:, vc, lo_s:hi_s],
                          w_rep[:, INIT_OI:INIT_OI + 1],
                      )
                      # Pool has no per-partition-scalar op (walrus rejects
                      # TensorScalarPtr on Pool): DVE pre-scales into a tmp
                      # (4x mode) and Pool does the accumulate
                      for oi in POOL_OFFS:
                          o = OFFSETS[oi]
                          lo = max(lo_s, o)
                          if lo >= hi_s:
                              continue
                          tmp = tmppool.tile([P, 512], BF16, tag="tmp")
                          nc.vector.tensor_scalar_mul(
                              tmp[:, 0:hi_s - lo],
                              field[:, vc, lo - o:hi_s - o],
                              w_rep[:, oi:oi + 1],
                          )
                          nc.gpsimd.tensor_tensor(
                              out=acc[:, vc, lo:hi_s], in0=acc[:, vc, lo:hi_s],
                              in1=tmp[:, 0:hi_s - lo], op=mybir.AluOpType.add,
                          )
                      for oi in ACT_OFFS:
                          o = OFFSETS[oi]
                          lo = max(lo_s, o)
                          tmp = tmppool.tile([P, 512], BF16, tag="tmp")
                          nc.scalar.activation(
                              out=tmp[:, 0:hi_s - lo], in_=field[:, vc, lo - o:hi_s - o],
                              func=mybir.ActivationFunctionType.Copy,
                              bias=0.0, scale=w_rep[:, oi:oi + 1],
                          )
                          nc.vector.tensor_tensor(
                              out=acc[:, vc, lo:hi_s], in0=acc[:, vc, lo:hi_s],
                              in1=tmp[:, 0:hi_s - lo],
                              op=mybir.AluOpType.add,
                          )
                      for oi in DVE_OFFS:
                          o = OFFSETS[oi]
                          lo = max(lo_s, o)
                          if lo >= hi_s:
                              continue
                          nc.vector.scalar_tensor_tensor(
                              out=acc[:, vc, lo:hi_s],
                              in0=field[:, vc, lo - o:hi_s - o],
                              scalar=w_rep[:, oi:oi + 1],
                              in1=acc[:, vc, lo:hi_s],
                              op0=mybir.AluOpType.mult, op1=mybir.AluOpType.add,
                          )

              # ============ gate phase (d-major channels, fp8 DR) =========
              # gate reuses the x_sb buffer (x dead after the v matmuls)
              gate = xpool.tile([P, CH, N], BF16, tag="xg")
              for gc in range(CH):
                  for nch in range(NCK):
                      ns = 512 * nch
                      ps = pspool.tile([P, 512], F32, tag="mm")
                      for ic in range(4):
                          nc.tensor.matmul(
                              ps[:, :],
                              lhsT=wg8_sb[:, ic, :, P * gc:P * (gc + 1)],
                              rhs=x8_sb[:, ic, :, ns:ns + 512],
                              perf_mode=mybir.MatmulPerfMode.DoubleRow,
                              start=(ic == 0), stop=(ic == 3),
                          )
                      with tc.high_priority():
                          nc.scalar.activation(
                              out=gate[:, gc, ns:ns + 512], in_=ps[:, :],
                              func=mybir.ActivationFunctionType.Sigmoid,
                              bias=bg_sb[:, gc:gc + 1], scale=1.0,
                          )

              # ==== coupling chain + PE conv offsets + gated, segment-outer =
              # gated multiplies the chain PSUM into the gate tile IN PLACE
              # (each gate element is consumed exactly once), so the out
              # projection for segment s can start right after segment s's
              # chains — no end-of-kernel projection tail.
              gated = gate
              for seg in range(NCK):
                  ns = 512 * seg
                  for vc in range(CH):
                      ps = pspool.tile([P, 512], F32, tag="mm")
                      if seg == 0:
                          # low cols are covered only by the coupling matmul,
                          # so it must open the chain (start=True)
                          nc.tensor.matmul(
                              ps[:, :], lhsT=g_sb[:, :],
                              rhs=acc[:, vc, ns:ns + 512],
                              start=True, stop=False,
                          )
                          for gi, oi in enumerate(PE_OFFS):
                              o = OFFSETS[oi]
                              lo = max(0, o - ns)
                              nc.tensor.matmul(
                                  ps[:, lo:512],
                                  lhsT=gdiagw[:, gi, :],
                                  rhs=field[:, vc, ns + lo - o:ns + 512 - o],
                                  start=False, stop=(gi == len(PE_OFFS) - 1),
                              )
                      else:
                          # conv offsets first (they need only field, so the
                          # scheduler can hoist them into earlier PE idle);
                          # the coupling matmul closes the chain once
                          # acc[:, vc, seg] is ready
                          for gi, oi in enumerate(PE_OFFS):
                              nc.tensor.matmul(
                                  ps[:, :],
                                  lhsT=gdiagw[:, gi, :],
                                  rhs=field[:, vc, ns - OFFSETS[oi]:
                                             ns + 512 - OFFSETS[oi]],
                                  start=(gi == 0), stop=False,
                              )
                          nc.tensor.matmul(
                              ps[:, :], lhsT=g_sb[:, :],
                              rhs=acc[:, vc, ns:ns + 512],
                              start=False, stop=True,
                          )
                      with tc.high_priority():
                          nc.vector.tensor_tensor(
                              out=gated[:, vc, ns:ns + 512], in0=ps[:, :],
                              in1=gate[:, vc, ns:ns + 512], op=mybir.AluOpType.mult,
                          )
                  # ---- out projection for this segment's 4 token tiles ----
                  for nt in range(4 * seg, 4 * seg + 4):
                      ystg = ypool.tile([P, D], F32, tag="y")
                      for fch in range(2):
                          fs = 512 * fch
                          ps = pspool.tile([P, 512], F32, tag="mm")
                          for vc in range(CH):
                              nc.tensor.matmul(
                                  ps[:, :],
                                  lhsT=gated[:, vc, P * nt:P * (nt + 1)],
                                  rhs=wo_sb[:, vc, fs:fs + 512],
                                  start=(vc == 0), stop=(vc == CH - 1),
                              )
                          with tc.high_priority():
                              if ob_zero:
                                  nc.scalar.activation(
                                      out=ystg[:, fs:fs + 512], in_=ps[:, :],
                                      func=mybir.ActivationFunctionType.Copy,
                                  )
                              else:
                                  nc.vector.tensor_add(
                                      ystg[:, fs:fs + 512], ps[:, :],
                                      ob_sb[:, fs:fs + 512],
                                  )
                      nc.sync.dma_start(out=y_d[P * nt:P * (nt + 1), :], in_=ystg[:, :])

    nc.compile()
    return nc


def _prep_shared(qkv_w, qkv_b, out_w, out_b, gate_w, gate_b, scale_gain, field_coupling):
    perm = PERM
    wk8 = np.ascontiguousarray(
        qkv_w[D:2 * D, :].T.reshape(4, P, 2, D).transpose(1, 0, 2, 3)
        .astype(NP_FP8))
    wg8 = np.ascontiguousarray(
        gate_w[perm, :].T.reshape(4, P, 2, D).transpose(1, 0, 2, 3)
        .astype(NP_FP8))
    wv = np.ascontiguousarray(qkv_w[2 * D:3 * D, :][perm, :].T.astype(NP_BF16))
    wo = np.ascontiguousarray(out_w[:, perm].T.astype(NP_BF16))
    bk = np.ascontiguousarray(qkv_b[D:2 * D].reshape(CH, P).T.astype(np.float32))
    bv = np.ascontiguousarray(qkv_b[2 * D:3 * D][perm].reshape(CH, P).T.astype(np.float32))
    bg = np.ascontiguousarray(gate_b[perm].reshape(CH, P).T.astype(np.float32))
    ob = np.ascontiguousarray(np.broadcast_to(out_b.astype(np.float32), (P, D)))
    sg = np.ascontiguousarray(scale_gain.astype(np.float32))
    fc = np.ascontiguousarray(field_coupling.astype(np.float32))
    return {"wk8": wk8, "wg8": wg8, "wv": wv, "wo": wo, "bk": bk,
            "bv": bv, "bg": bg, "ob": ob, "sg": sg, "fc": fc}


def _make_in_maps(x, shared, ob_zero=True):
    in_maps = []
    for b in range(B):
        m = dict(shared)
        if ob_zero:
            m.pop("ob", None)
        xt = x[b].T
        m["x_cm"] = np.ascontiguousarray(xt.astype(NP_BF16))
        m["x8"] = np.ascontiguousarray(
            xt.reshape(4, P, 2, N).transpose(1, 0, 2, 3).astype(NP_FP8))
        in_maps.append(m)
    return in_maps


def kernel(x, qkv_w, qkv_b, out_w, out_b, gate_w, gate_b, scale_gain,
           field_coupling):
    x = np.asarray(x, dtype=np.float32)
    qkv_w = np.asarray(qkv_w, dtype=np.float32)
    qkv_b = np.asarray(qkv_b, dtype=np.float32)
    out_w = np.asarray(out_w, dtype=np.float32)
    out_b = np.asarray(out_b, dtype=np.float32)
    gate_w = np.asarray(gate_w, dtype=np.float32)
    gate_b = np.asarray(gate_b, dtype=np.float32)
    scale_gain = np.asarray(scale_gain, dtype=np.float32)
    field_coupling = np.asarray(field_coupling, dtype=np.float32)

    ob_zero = not np.any(out_b)
    key = ("nc", ob_zero)
    if key not in _CACHE:
        _CACHE[key] = _build_program(ob_zero=ob_zero)
    nc = _CACHE[key]

    shared = _prep_shared(qkv_w, qkv_b, out_w, out_b, gate_w, gate_b,
                          scale_gain, field_coupling)
    in_maps = _make_in_maps(x, shared, ob_zero=ob_zero)

    res = bass_utils.run_bass_kernel_spmd(nc, in_maps, list(range(NCORES)))
    out = np.stack([np.asarray(res.results[b]["y"], dtype=np.float32)
                    for b in range(B)], axis=0)
    return out
